# revision 22
# baseline (speedup 1.0000x reference)
"""Self-contained Trainium2 kernel for nn_AttentionEntryPoint (dense attention
with BatchNorm input norm and softmax over the batch axis), batch-sharded
across 8 NeuronCores.

Key structure (v2):
- scores = xn (Q K^T/sqrt(C)) xn^T: G = Q K^T is precomputed on the PE during
  the collective entry-barrier / stats-AllReduce stall (no stats dependency),
  so only xn -> yT = G-projection -> scores(chunk0) sit on the critical path
  before the first big AllReduce trigger. The separate q/k projections of the
  baseline are gone.
- The cross-batch softmax max is AllReduced as int8 (scores/32 rounded):
  any consistent shift within ~85 of the true max is mathematically exact for
  softmax and safe for exp range, and |scores| < 4064 fits int8*32. This
  halves the max-AR bytes. Dequant is folded into exp via activation scale.
- v, xnT and the second q-chunk work are scheduled under the AllReduce
  windows where the PE would otherwise idle.

kernel(**inputs) takes the full inputs (x [16,512,32,32], Q/K/V [512,512])
and returns the full output [16, 1024, 512] (attn + xn, as in the reference).
"""
import sys
import numpy as np

for _p in ("/opt/trn_rl_repo",):
    if _p not in sys.path:
        sys.path.append(_p)

import jax
from jax.sharding import Mesh, PartitionSpec, NamedSharding
from jax.experimental.shard_map import shard_map

import concourse.bass as bass
import concourse.bacc as bacc
import concourse.mybir as mybir
import concourse.tile as tile
from concourse import bass2jax
from concourse.bass2jax import _bass_exec_p, install_neuronx_cc_hook
from concourse.masks import make_identity

F32 = mybir.dt.float32
F32R = mybir.dt.float32r
BF16 = mybir.dt.bfloat16
I8 = mybir.dt.int8
AX = mybir.AxisListType
OP = mybir.AluOpType
ACT = mybir.ActivationFunctionType

N_CORES = 8
EPS = 1e-5
MQ_SCALE = 32.0  # int8 max quantization step; covers |scores| <= 4064


def build_nc(C=512, S=1024, b_loc=2, num_cores=N_CORES, warmup=16, g_chunks=2,
             use_int8_max=True):
    CT = C // 128          # channel tiles (contraction side of projections)
    DT = C // 128          # d tiles
    ST = S // 128          # s tiles
    ET = C // 128          # e tiles (contraction of G = Q K^T)
    G = g_chunks
    QC = S // G            # q chunk width
    QT = QC // 128         # q tiles per chunk
    NB = num_cores * b_loc * S   # batchnorm sample count (B*S)
    WS = min(512, S)             # matmul moving width for S-wide passes
    WQ = min(512, QC)            # matmul moving width within a q chunk
    NSH = S // WS
    ISQ = float(1.0 / np.sqrt(np.float32(C)))
    MF = F32R
    MAXDT = I8 if use_int8_max else BF16

    nc = bacc.Bacc("TRN2", target_bir_lowering=False, num_devices=num_cores)
    rg = [list(range(num_cores))]

    x_ext = nc.dram_tensor("x", [b_loc, C, S], F32, kind="ExternalInput").ap()
    q_ext = nc.dram_tensor("Q", [C, C], MF, kind="ExternalInput").ap()
    k_ext = nc.dram_tensor("K", [C, C], MF, kind="ExternalInput").ap()
    v_ext = nc.dram_tensor("V", [C, C], MF, kind="ExternalInput").ap()
    out_ext = nc.dram_tensor("out", [b_loc, S, C], F32, kind="ExternalOutput").ap()

    stats_in = nc.dram_tensor("stats_in", [128, 2 * CT], F32)
    stats_out = nc.dram_tensor("stats_out", [128, 2 * CT], F32, addr_space="Shared")
    max_in = [nc.dram_tensor(f"max_in{g}", [128, ST * QC], MAXDT) for g in range(G)]
    max_out = [nc.dram_tensor(f"max_out{g}", [128, ST * QC], MAXDT,
                              addr_space="Shared") for g in range(G)]
    sum_in = [nc.dram_tensor(f"sum_in{g}", [128, ST * QC], BF16) for g in range(G)]
    sum_out = [nc.dram_tensor(f"sum_out{g}", [128, ST * QC], BF16,
                              addr_space="Shared") for g in range(G)]

    with tile.TileContext(nc) as tc:
        p_small = tc.alloc_tile_pool(name="small", bufs=1)
        p_x = tc.alloc_tile_pool(name="xdata", bufs=1)
        p_vw = tc.alloc_tile_pool(name="vw", bufs=1, side="right")
        p_g = tc.alloc_tile_pool(name="gmat", bufs=1, side="right")
        p_qk = tc.alloc_tile_pool(name="qkw", bufs=1, side="right")
        p_mm = tc.alloc_tile_pool(name="psum_mm", bufs=4, space="PSUM")
        p_tr = tc.alloc_tile_pool(name="psum_tr", bufs=2, space="PSUM")

        kwseq = [0]

        def keepwarm(dep, n=2):
            # dummy matmuls reading `dep` to keep the PE HAM clock warm
            w = min(512, dep.shape[1])
            for i in range(n):
                kwseq[0] += 1
                pw = p_mm.tile([128, w], F32, tag="mm", name=f"kw{kwseq[0]}")
                nc.tensor.matmul(pw[:], dep[:, :128], dep[:, :w],
                                 start=True, stop=True)

        # ---- constants / warmup ----------------------------------------
        identf = p_small.tile([128, 128], F32, tag="identf", name="identf")
        make_identity(nc, identf[:])
        ident = p_small.tile([128, 128], MF, tag="ident", name="ident")
        nc.vector.tensor_copy(ident[:], identf[:])
        for i in range(warmup):
            pw = p_mm.tile([128, 128], F32, tag="mm", name=f"wup{i}")
            nc.tensor.matmul(pw[:], (ident[:]), (ident[:]),
                             start=True, stop=True)

        # ---- loads (x first: stats path is latency-critical) -----------
        xt = [p_x.tile([128, CT * S], F32, tag=f"x{b}", name=f"x{b}")
              for b in range(b_loc)]
        for b in range(b_loc):
            for ct in range(CT):
                nc.sync.dma_start(
                    out=xt[b][:, ct * S:(ct + 1) * S],
                    in_=x_ext[b, ct * 128:(ct + 1) * 128, :])
        qw = p_qk.tile([128, CT * C], MF, tag="qw", name="qw")
        kw = p_qk.tile([128, CT * C], MF, tag="kw", name="kw")
        vw = p_vw.tile([128, CT * C], MF, tag="vw", name="vw")
        for wt, ext in ((qw, q_ext), (kw, k_ext), (vw, v_ext)):
            nc.sync.dma_start(
                out=wt[:].rearrange("p (ct d) -> p ct d", ct=CT),
                in_=ext.rearrange("(ct p) d -> p ct d", p=128))

        # ---- batchnorm stats + AllReduce -------------------------------
        stats = p_small.tile([128, 2 * CT], F32, tag="stats", name="stats")
        xsum = [[p_small.tile([128, 1], F32, tag=f"xs{b}_{ct}", name=f"xs{b}_{ct}")
                 for ct in range(CT)] for b in range(b_loc)]
        xsq = [[p_small.tile([128, 1], F32, tag=f"xq{b}_{ct}", name=f"xq{b}_{ct}")
                for ct in range(CT)] for b in range(b_loc)]
        p_scr = tc.alloc_tile_pool(name="sqscr", bufs=2)
        for b in range(b_loc):
            for ct in range(CT):
                xs = xt[b][:, ct * S:(ct + 1) * S]
                nc.vector.tensor_reduce(xsum[b][ct][:], xs, AX.X, OP.add)
                sq = p_scr.tile([128, S], F32, tag="sqscr", name=f"sq{b}_{ct}")
                nc.scalar.activation(sq[:], xs, ACT.Square, accum_out=xsq[b][ct][:])
        p_scr.release()
        for ct in range(CT):
            nc.vector.tensor_tensor(stats[:, ct:ct + 1],
                                    xsum[0][ct][:], xsum[1][ct][:], OP.add)
            nc.vector.tensor_tensor(stats[:, CT + ct:CT + ct + 1],
                                    xsq[0][ct][:], xsq[1][ct][:], OP.add)
        nc.sync.dma_start(out=stats_in[:, :], in_=stats[:])
        nc.gpsimd.collective_compute(
            "AllReduce", OP.add, replica_groups=rg,
            ins=[stats_in.ap().opt()], outs=[stats_out.ap().opt()])
        statsg = p_small.tile([128, 2 * CT], F32, tag="statsg", name="statsg")
        nc.sync.dma_start(out=statsg[:], in_=stats_out[:, :])

        # ---- G = (Q K^T)/sqrt(C) during the stats stall ----------------
        # QT[e, c] = Q[c, e], KT[e, d] = K[d, e] via PE transposes.
        qT = p_qk.tile([128, ET * C], MF, tag="qT", name="qT")
        kTt = p_qk.tile([128, ET * C], MF, tag="kTt", name="kTt")
        for src, dst, nm in ((qw, qT, "q"), (kw, kTt, "k")):
            for ct in range(CT):
                for et in range(ET):
                    pt = p_tr.tile([128, 128], MF, tag="tr", name=f"tr{nm}{ct}_{et}")
                    nc.tensor.transpose(
                        pt[:], src[:, ct * C + et * 128: ct * C + (et + 1) * 128],
                        ident[:])
                    nc.any.tensor_copy(
                        dst[:, et * C + ct * 128: et * C + (ct + 1) * 128], pt[:])
        gmat = p_g.tile([128, CT * C], MF, tag="gmat", name="gmat")
        for ct in range(CT):
            ps = p_mm.tile([128, C], F32, tag="mm", name=f"psg{ct}")
            for et in range(ET):
                nc.tensor.matmul(
                    ps[:],
                    qT[:, et * C + ct * 128: et * C + (ct + 1) * 128],
                    kTt[:, et * C:(et + 1) * C],
                    start=(et == 0), stop=(et == ET - 1))
            nc.scalar.mul(gmat[:, ct * C:(ct + 1) * C], ps[:], ISQ)
        p_qk.release()
        keepwarm(xt[0][:], n=8)
        keepwarm(xt[1][:], n=8)

        # ---- scale/bias from global stats (wide [128, CT] ops) ---------
        epsb = p_small.tile([128, 1], F32, tag="epsb", name="epsb")
        nc.vector.memset(epsb[:], EPS)
        mean = p_small.tile([128, CT], F32, tag="mean", name="mean")
        nc.scalar.mul(mean[:], statsg[:, 0:CT], 1.0 / NB)
        ex2 = p_small.tile([128, CT], F32, tag="ex2", name="ex2")
        nc.scalar.mul(ex2[:], statsg[:, CT:2 * CT], 1.0 / NB)
        var = p_small.tile([128, CT], F32, tag="var", name="var")
        nc.vector.scalar_tensor_tensor(var[:], mean[:], 1.0, mean[:],
                                       OP.mult, OP.mult)
        nc.vector.tensor_tensor(var[:], ex2[:], var[:], OP.subtract)
        std = p_small.tile([128, CT], F32, tag="std", name="std")
        nc.scalar.activation(std[:], var[:], ACT.Sqrt, bias=epsb[:])
        scale = p_small.tile([128, CT], F32, tag="scale", name="scale")
        nc.vector.reciprocal(scale[:], std[:])
        nbias = p_small.tile([128, CT], F32, tag="nbias", name="nbias")
        nc.vector.scalar_tensor_tensor(nbias[:], mean[:], -1.0, scale[:],
                                       OP.mult, OP.mult)

        # ---- xn = x*scale + nbias (split scalar/vector) ----------------
        p_xn = tc.alloc_tile_pool(name="xn", bufs=1, side="right")
        xn = [p_xn.tile([128, CT * S], MF, tag=f"xn{b}", name=f"xn{b}")
              for b in range(b_loc)]
        for b in range(b_loc):
            for ct in range(CT):
                osl = xn[b][:, ct * S:(ct + 1) * S]
                isl = xt[b][:, ct * S:(ct + 1) * S]
                if (b * CT + ct) % 2 == 0:
                    nc.scalar.activation(osl, isl, ACT.Identity,
                                         bias=nbias[:, ct:ct + 1],
                                         scale=scale[:, ct:ct + 1])
                else:
                    nc.vector.tensor_scalar(osl, isl, scale[:, ct:ct + 1],
                                            nbias[:, ct:ct + 1],
                                            OP.mult, OP.add)
        keepwarm(xn[0][:], n=2)
        p_x.release()

        # ---- yT[d, s] = sum_c G[c, d] xn[c, s] -------------------------
        p_yt = tc.alloc_tile_pool(name="yt", bufs=1, side="right")
        yT = [p_yt.tile([128, DT * S], MF, tag=f"yT{b}", name=f"yT{b}")
              for b in range(b_loc)]
        # sh (q-range) outermost: all dt tiles of chunk-0's columns finish
        # first so the scores0 matmuls can start while chunk-1's yT runs
        for sh in range(NSH):
            pss = [[p_mm.tile([128, WS], F32, tag="mm", name=f"psy{b}_{dt}_{sh}")
                    for dt in range(DT)] for b in range(b_loc)]
            for dt in range(DT):
                for ct in range(CT):
                    for b in range(b_loc):
                        nc.tensor.matmul(
                            pss[b][dt][:],
                            gmat[:, ct * C + dt * 128: ct * C + (dt + 1) * 128],
                            xn[b][:, ct * S + sh * WS: ct * S + (sh + 1) * WS],
                            start=(ct == 0), stop=(ct == CT - 1))
                for b in range(b_loc):
                    nc.any.tensor_copy(
                        yT[b][:, dt * S + sh * WS: dt * S + (sh + 1) * WS],
                        pss[b][dt][:])

        # ---- scores + softmax pipeline ---------------------------------
        p_sc = tc.alloc_tile_pool(name="scores", bufs=1)
        p_soft = tc.alloc_tile_pool(name="soft", bufs=1)
        scT = [[p_sc.tile([128, ST * QC], F32, tag=f"sc{g}_{b}", name=f"sc{g}_{b}")
                for b in range(b_loc)] for g in range(G)]
        mq = [p_soft.tile([128, ST * QC], MAXDT, tag="mqbuf", name=f"mq{g}",
                          bufs=1) for g in range(G)]
        mgs = []

        def scores_chunk(g):
            for kt in range(ST):
                for b in range(b_loc):
                    for qh in range(QC // WQ):
                        ps = p_mm.tile([128, WQ], F32, tag="mm",
                                       name=f"pss{g}_{kt}_{b}_{qh}")
                        for dt in range(DT):
                            nc.tensor.matmul(
                                ps[:],
                                xn[b][:, dt * S + kt * 128: dt * S + (kt + 1) * 128],
                                yT[b][:, dt * S + g * QC + qh * WQ: dt * S + g * QC + (qh + 1) * WQ],
                                start=(dt == 0), stop=(dt == DT - 1))
                        dsl = scT[g][b][:, kt * QC + qh * WQ: kt * QC + (qh + 1) * WQ]
                        if (kt + b) % 2 == 0:
                            nc.scalar.copy(dsl, ps[:])
                        else:
                            nc.vector.tensor_copy(dsl, ps[:])
                ksl = slice(kt * QC, (kt + 1) * QC)
                mscr = p_soft.tile([128, QC], F32, tag="mscr",
                                   name=f"mscr{g}_{kt}", bufs=2)
                nc.vector.tensor_tensor(mscr[:], scT[g][0][:, ksl],
                                        scT[g][1][:, ksl], OP.max)
                if use_int8_max:
                    nc.vector.tensor_scalar_mul(mq[g][:, ksl], mscr[:],
                                                1.0 / MQ_SCALE)
                else:
                    nc.vector.tensor_copy(mq[g][:, ksl], mscr[:])
            nc.sync.dma_start(out=max_in[g][:, :], in_=mq[g][:])
            keepwarm(yT[0][:], n=2)
            nc.gpsimd.collective_compute(
                "AllReduce", OP.max, replica_groups=rg,
                ins=[max_in[g].ap().opt()], outs=[max_out[g].ap().opt()])

        scores_chunk(0)
        scores_chunk(1)
        p_yt.release()

        zgs = []

        def softmax_chunk(g):
            # load the max-AR result on the vector queue: the first consumer
            # (the subtract) is a vector op anyway, so no queue is blocked
            mg = p_soft.tile([128, ST * QC], MAXDT, tag="mgbuf", name=f"mg{g}",
                             bufs=1)
            nc.gpsimd.dma_start(out=mg[:], in_=max_out[g][:, :])
            keepwarm(scT[g][0][:], n=2)
            zloc = p_soft.tile([128, ST * QC], BF16, tag="zloc", name=f"zloc{g}",
                               bufs=1)
            for kt2 in range(ST // 2):
                ksl = slice(kt2 * 2 * QC, (kt2 + 1) * 2 * QC)
                for b in range(b_loc):
                    if use_int8_max:
                        nc.vector.scalar_tensor_tensor(
                            scT[g][b][:, ksl], scT[g][b][:, ksl], 1.0 / MQ_SCALE,
                            mg[:, ksl], OP.mult, OP.subtract)
                        nc.scalar.activation(scT[g][b][:, ksl], scT[g][b][:, ksl],
                                             ACT.Exp, scale=MQ_SCALE)
                    else:
                        nc.vector.scalar_tensor_tensor(
                            scT[g][b][:, ksl], scT[g][b][:, ksl], 1.0,
                            mg[:, ksl], OP.mult, OP.subtract)
                        nc.scalar.activation(scT[g][b][:, ksl], scT[g][b][:, ksl],
                                             ACT.Exp)
                nc.vector.tensor_tensor(zloc[:, ksl], scT[g][0][:, ksl],
                                        scT[g][1][:, ksl], OP.add)
            nc.sync.dma_start(out=sum_in[g][:, :], in_=zloc[:])
            nc.gpsimd.collective_compute(
                "AllReduce", OP.add, replica_groups=rg,
                ins=[sum_in[g].ap().opt()], outs=[sum_out[g].ap().opt()])
            keepwarm(zloc[:], n=2)

        softmax_chunk(0)

        # ---- v = xn @ V (bf16), scheduled under the AR windows ---------
        p_v = tc.alloc_tile_pool(name="vv", bufs=1)
        vt = [p_v.tile([128, ST * C], BF16, tag=f"v{b}", name=f"v{b}")
              for b in range(b_loc)]
        for b in range(b_loc):
            for st in range(ST):
                ps = p_mm.tile([128, C], F32, tag="mm", name=f"psv{b}_{st}")
                for ct in range(CT):
                    nc.tensor.matmul(
                        ps[:],
                        xn[b][:, ct * S + st * 128: ct * S + (st + 1) * 128],
                        vw[:, ct * C: (ct + 1) * C],
                        start=(ct == 0), stop=(ct == CT - 1))
                nc.any.tensor_copy(vt[b][:, st * C:(st + 1) * C], ps[:])

        softmax_chunk(1)
        keepwarm(scT[1][0][:], n=6)
        # ---- xnT (bf16) via PE transposes (no DMA: slow strided DMA
        # transposes poison the shared DMA-completion semaphores that the
        # collective triggers wait on) ---------------------------------
        xnT = [p_v.tile([128, ST * C], BF16, tag=f"xnT{b}", name=f"xnT{b}")
               for b in range(b_loc)]
        for b in range(b_loc):
            for ct in range(CT):
                for st in range(ST):
                    pt = p_tr.tile([128, 128], MF, tag="tr",
                                   name=f"trx{b}_{ct}_{st}")
                    nc.tensor.transpose(
                        pt[:],
                        xn[b][:, ct * S + st * 128: ct * S + (st + 1) * 128],
                        ident[:])
                    dst = xnT[b][:, st * C + ct * 128: st * C + (ct + 1) * 128]
                    if (ct + st) % 2 == 0:
                        nc.vector.tensor_copy(dst, pt[:])
                    else:
                        nc.scalar.copy(dst, pt[:])

        p_xn.release()
        p_g.release()
        p_vw.release()

        # ---- normalize, attention, residual, store ---------------------
        p_att = tc.alloc_tile_pool(name="psum_attn", bufs=2, space="PSUM")
        p_rz = tc.alloc_tile_pool(name="rz", bufs=1, side="right")
        p_out = tc.alloc_tile_pool(name="outs", bufs=2, side="right")

        def attn_chunk(g):
            zg = p_soft.tile([128, ST * QC], BF16, tag="zgbuf", name=f"zg{g}",
                             bufs=1)
            nc.gpsimd.dma_start(out=zg[:], in_=sum_out[g][:, :])
            zgs.append(zg)
            keepwarm(zgs[g][:], n=2)
            rz = p_rz.tile([128, ST * QC], F32, tag="rz", name=f"rz{g}")
            wb = [p_soft.tile([128, ST * QC], BF16, tag="wbbuf",
                              name=f"wb{g}_{b}", bufs=2) for b in range(b_loc)]
            # sliced so the first attn matmuls can start as soon as the first
            # kt-group of weights is normalized
            for kth in range(ST // 2):
                ksl = slice(kth * 2 * QC, (kth + 1) * 2 * QC)
                zgf = p_rz.tile([128, 2 * QC], F32, tag="zgf",
                                name=f"zgf{g}_{kth}", bufs=2)
                nc.scalar.copy(zgf[:], zg[:, ksl])
                nc.vector.reciprocal_approx_fast(out=rz[:, ksl], in_=zgf[:])
                for b in range(b_loc):
                    nc.vector.tensor_tensor(wb[b][:, ksl], scT[g][b][:, ksl],
                                            rz[:, ksl], OP.mult)
            for b in range(b_loc):
                ot = p_out.tile([128, QT * C], F32, tag="ot", name=f"ot{g}_{b}")
                for qt in range(QT):
                    st_glob = g * QT + qt
                    pa = p_att.tile([128, C], F32, tag="attn", name=f"pa{g}_{b}_{qt}")
                    for kt in range(ST):
                        wsl = wb[b][:, kt * QC + qt * 128: kt * QC + (qt + 1) * 128]
                        nc.tensor.matmul(
                            pa[:], wsl, vt[b][:, kt * C:(kt + 1) * C],
                            start=(kt == 0), stop=(kt == ST - 1))
                    nc.vector.scalar_tensor_tensor(
                        ot[:, qt * C:(qt + 1) * C], pa[:], 1.0,
                        xnT[b][:, st_glob * C:(st_glob + 1) * C],
                        OP.mult, OP.add)
                    nc.sync.dma_start(
                        out=out_ext[b, g * QC + qt * 128: g * QC + (qt + 1) * 128, :],
                        in_=ot[:, qt * C:(qt + 1) * C])

        attn_chunk(0)
        attn_chunk(1)

        for p in (p_out, p_rz, p_att, p_v, p_soft, p_sc, p_small, p_tr, p_mm):
            p.release()

    nc.compile()
    return nc


def _round_f32r(a):
    """Round-to-nearest-even to the FP32R encoding (11-bit mantissa)."""
    bits = np.ascontiguousarray(a, dtype=np.float32).view(np.uint32).copy()
    low = bits & np.uint32(0xFFF)
    bit12 = (bits >> np.uint32(12)) & np.uint32(1)
    up = (low > 0x800) | ((low == 0x800) & (bit12 == 1))
    bits = (bits & ~np.uint32(0xFFF)) + (up.astype(np.uint32) << np.uint32(12))
    return bits.view(np.float32)


def make_runner(nc, n_cores):
    install_neuronx_cc_hook()
    partition_name = nc.partition_id_tensor.name if nc.partition_id_tensor else None
    in_names, out_names, out_avals, zero_shapes = [], [], [], []
    for alloc in nc.m.functions[0].allocations:
        if not isinstance(alloc, mybir.MemoryLocationSet):
            continue
        name = alloc.memorylocations[0].name
        if alloc.kind == "ExternalInput":
            if name != partition_name:
                in_names.append(name)
        elif alloc.kind == "ExternalOutput":
            out_names.append(name)
            shape = tuple(alloc.tensor_shape)
            dtype = mybir.dt.np(alloc.dtype)
            out_avals.append(jax.core.ShapedArray(shape, dtype))
            zero_shapes.append((shape, dtype))
    n_params = len(in_names)
    n_outs = len(out_avals)
    all_in_names = list(in_names) + list(out_names)
    if partition_name is not None:
        all_in_names.append(partition_name)

    def _body(*args):
        operands = list(args)
        if partition_name is not None:
            operands.append(bass2jax.partition_id_tensor())
        outs = _bass_exec_p.bind(
            *operands,
            out_avals=tuple(out_avals),
            in_names=tuple(all_in_names),
            out_names=tuple(out_names),
            lowering_input_output_aliases=(),
            sim_require_finite=True,
            sim_require_nnan=True,
            nc=nc,
        )
        return tuple(outs)

    devices = jax.devices()[:n_cores]
    mesh = Mesh(np.asarray(devices), ("core",))
    in_specs = (PartitionSpec("core"),) * (n_params + n_outs)
    out_specs = (PartitionSpec("core"),) * n_outs
    sharded = jax.jit(
        shard_map(_body, mesh=mesh, in_specs=in_specs, out_specs=out_specs,
                  check_rep=False),
        donate_argnums=tuple(range(n_params, n_params + n_outs)),
        keep_unused=True,
    )
    shd = NamedSharding(mesh, PartitionSpec("core"))

    def run(in_maps):
        concat_in = [
            np.concatenate([np.asarray(in_maps[c][name]) for c in range(n_cores)],
                           axis=0)
            for name in in_names
        ]
        concat_zeros = [
            np.zeros((n_cores * s[0], *s[1:]), d) for (s, d) in zero_shapes
        ]
        # pre-place on devices, wait for transfers to complete
        placed = [jax.device_put(a, shd) for a in concat_in + concat_zeros]
        jax.block_until_ready(placed)
        out_arrs = sharded(*placed)
        jax.block_until_ready(out_arrs)
        return [
            {name: np.asarray(out_arrs[i]).reshape(n_cores, *out_avals[i].shape)[c]
             for i, name in enumerate(out_names)}
            for c in range(n_cores)
        ]

    return run


_CACHE = {}


def kernel(x, Q, K, V):
    B, C, H, W = x.shape
    S = H * W
    b_loc = B // N_CORES
    key = (B, C, S)
    if key not in _CACHE:
        nc = build_nc(C=C, S=S, b_loc=b_loc)
        _CACHE[key] = make_runner(nc, N_CORES)
    run = _CACHE[key]

    xr = np.ascontiguousarray(np.asarray(x, dtype=np.float32).reshape(B, C, S))
    Qr, Kr, Vr = _round_f32r(Q), _round_f32r(K), _round_f32r(V)
    in_maps = [{"x": xr[b_loc * i: b_loc * (i + 1)], "Q": Qr, "K": Kr, "V": Vr}
               for i in range(N_CORES)]
    results = run(in_maps)
    out = np.concatenate(
        [results[i]["out"].reshape(b_loc, S, C) for i in range(N_CORES)], axis=0)
    return out.astype(np.float32)


# revision 23
# speedup vs baseline: 1.0231x; 1.0231x over previous
"""Self-contained Trainium2 kernel for nn_AttentionEntryPoint (dense attention
with BatchNorm input norm and softmax over the batch axis), batch-sharded
across 8 NeuronCores.

Key structure (v2):
- scores = xn (Q K^T/sqrt(C)) xn^T: G = Q K^T is precomputed on the PE during
  the collective entry-barrier / stats-AllReduce stall (no stats dependency),
  so only xn -> yT = G-projection -> scores(chunk0) sit on the critical path
  before the first big AllReduce trigger. The separate q/k projections of the
  baseline are gone.
- The cross-batch softmax max is AllReduced as int8 (scores/32 rounded):
  any consistent shift within ~85 of the true max is mathematically exact for
  softmax and safe for exp range, and |scores| < 4064 fits int8*32. This
  halves the max-AR bytes. Dequant is folded into exp via activation scale.
- v, xnT and the second q-chunk work are scheduled under the AllReduce
  windows where the PE would otherwise idle.

kernel(**inputs) takes the full inputs (x [16,512,32,32], Q/K/V [512,512])
and returns the full output [16, 1024, 512] (attn + xn, as in the reference).
"""
import sys
import numpy as np

for _p in ("/opt/trn_rl_repo",):
    if _p not in sys.path:
        sys.path.append(_p)

import jax
from jax.sharding import Mesh, PartitionSpec, NamedSharding
from jax.experimental.shard_map import shard_map

import concourse.bass as bass
import concourse.bacc as bacc
import concourse.mybir as mybir
import concourse.tile as tile
from concourse import bass2jax
from concourse.bass2jax import _bass_exec_p, install_neuronx_cc_hook
from concourse.masks import make_identity

F32 = mybir.dt.float32
F32R = mybir.dt.float32r
BF16 = mybir.dt.bfloat16
I8 = mybir.dt.int8
AX = mybir.AxisListType
OP = mybir.AluOpType
ACT = mybir.ActivationFunctionType

N_CORES = 8
EPS = 1e-5
MQ_SCALE = 32.0  # int8 max quantization step; covers |scores| <= 4064


def build_nc(C=512, S=1024, b_loc=2, num_cores=N_CORES, warmup=16, g_chunks=2,
             use_int8_max=True):
    CT = C // 128          # channel tiles (contraction side of projections)
    DT = C // 128          # d tiles
    ST = S // 128          # s tiles
    ET = C // 128          # e tiles (contraction of G = Q K^T)
    G = g_chunks
    QC = S // G            # q chunk width
    QT = QC // 128         # q tiles per chunk
    NB = num_cores * b_loc * S   # batchnorm sample count (B*S)
    WS = min(512, S)             # matmul moving width for S-wide passes
    WQ = min(512, QC)            # matmul moving width within a q chunk
    NSH = S // WS
    ISQ = float(1.0 / np.sqrt(np.float32(C)))
    MF = F32R
    MAXDT = I8 if use_int8_max else BF16

    nc = bacc.Bacc("TRN2", target_bir_lowering=False, num_devices=num_cores)
    rg = [list(range(num_cores))]

    x_ext = nc.dram_tensor("x", [b_loc, C, S], F32, kind="ExternalInput").ap()
    q_ext = nc.dram_tensor("Q", [C, C], MF, kind="ExternalInput").ap()
    k_ext = nc.dram_tensor("K", [C, C], MF, kind="ExternalInput").ap()
    v_ext = nc.dram_tensor("V", [C, C], MF, kind="ExternalInput").ap()
    out_ext = nc.dram_tensor("out", [b_loc, S, C], F32, kind="ExternalOutput").ap()

    stats_in = nc.dram_tensor("stats_in", [128, 2 * CT], F32)
    stats_out = nc.dram_tensor("stats_out", [128, 2 * CT], F32, addr_space="Shared")
    max_in = [nc.dram_tensor(f"max_in{g}", [128, ST * QC], MAXDT) for g in range(G)]
    max_out = [nc.dram_tensor(f"max_out{g}", [128, ST * QC], MAXDT,
                              addr_space="Shared") for g in range(G)]
    sum_in = [nc.dram_tensor(f"sum_in{g}", [128, ST * QC], BF16) for g in range(G)]
    sum_out = [nc.dram_tensor(f"sum_out{g}", [128, ST * QC], BF16,
                              addr_space="Shared") for g in range(G)]

    with tile.TileContext(nc) as tc:
        p_small = tc.alloc_tile_pool(name="small", bufs=1)
        p_x = tc.alloc_tile_pool(name="xdata", bufs=1)
        p_vw = tc.alloc_tile_pool(name="vw", bufs=1, side="right")
        p_g = tc.alloc_tile_pool(name="gmat", bufs=1, side="right")
        p_qk = tc.alloc_tile_pool(name="qkw", bufs=1, side="right")
        p_mm = tc.alloc_tile_pool(name="psum_mm", bufs=4, space="PSUM")
        p_tr = tc.alloc_tile_pool(name="psum_tr", bufs=2, space="PSUM")

        kwseq = [0]

        def keepwarm(dep, n=2):
            # dummy matmuls reading `dep` to keep the PE HAM clock warm
            w = min(512, dep.shape[1])
            for i in range(n):
                kwseq[0] += 1
                pw = p_mm.tile([128, w], F32, tag="mm", name=f"kw{kwseq[0]}")
                nc.tensor.matmul(pw[:], dep[:, :128], dep[:, :w],
                                 start=True, stop=True)

        # ---- constants / warmup ----------------------------------------
        identf = p_small.tile([128, 128], F32, tag="identf", name="identf")
        make_identity(nc, identf[:])
        ident = p_small.tile([128, 128], MF, tag="ident", name="ident")
        nc.vector.tensor_copy(ident[:], identf[:])
        for i in range(warmup):
            pw = p_mm.tile([128, 128], F32, tag="mm", name=f"wup{i}")
            nc.tensor.matmul(pw[:], (ident[:]), (ident[:]),
                             start=True, stop=True)

        # ---- loads (x first: stats path is latency-critical) -----------
        xt = [p_x.tile([128, CT * S], F32, tag=f"x{b}", name=f"x{b}")
              for b in range(b_loc)]
        for b in range(b_loc):
            for ct in range(CT):
                nc.sync.dma_start(
                    out=xt[b][:, ct * S:(ct + 1) * S],
                    in_=x_ext[b, ct * 128:(ct + 1) * 128, :])
        qw = p_qk.tile([128, CT * C], MF, tag="qw", name="qw")
        kw = p_qk.tile([128, CT * C], MF, tag="kw", name="kw")
        vw = p_vw.tile([128, CT * C], MF, tag="vw", name="vw")
        for wt, ext in ((qw, q_ext), (kw, k_ext), (vw, v_ext)):
            nc.sync.dma_start(
                out=wt[:].rearrange("p (ct d) -> p ct d", ct=CT),
                in_=ext.rearrange("(ct p) d -> p ct d", p=128))

        # ---- batchnorm stats + AllReduce -------------------------------
        stats = p_small.tile([128, 2 * CT], F32, tag="stats", name="stats")
        xsum = [[p_small.tile([128, 1], F32, tag=f"xs{b}_{ct}", name=f"xs{b}_{ct}")
                 for ct in range(CT)] for b in range(b_loc)]
        xsq = [[p_small.tile([128, 1], F32, tag=f"xq{b}_{ct}", name=f"xq{b}_{ct}")
                for ct in range(CT)] for b in range(b_loc)]
        p_scr = tc.alloc_tile_pool(name="sqscr", bufs=2)
        for b in range(b_loc):
            for ct in range(CT):
                xs = xt[b][:, ct * S:(ct + 1) * S]
                nc.vector.tensor_reduce(xsum[b][ct][:], xs, AX.X, OP.add)
                sq = p_scr.tile([128, S], F32, tag="sqscr", name=f"sq{b}_{ct}")
                nc.scalar.activation(sq[:], xs, ACT.Square, accum_out=xsq[b][ct][:])
        p_scr.release()
        for ct in range(CT):
            nc.vector.tensor_tensor(stats[:, ct:ct + 1],
                                    xsum[0][ct][:], xsum[1][ct][:], OP.add)
            nc.vector.tensor_tensor(stats[:, CT + ct:CT + ct + 1],
                                    xsq[0][ct][:], xsq[1][ct][:], OP.add)
        nc.sync.dma_start(out=stats_in[:, :], in_=stats[:])
        nc.gpsimd.collective_compute(
            "AllReduce", OP.add, replica_groups=rg,
            ins=[stats_in.ap().opt()], outs=[stats_out.ap().opt()])
        statsg = p_small.tile([128, 2 * CT], F32, tag="statsg", name="statsg")
        nc.sync.dma_start(out=statsg[:], in_=stats_out[:, :])

        # ---- G = (Q K^T)/sqrt(C) during the stats stall ----------------
        # QT[e, c] = Q[c, e], KT[e, d] = K[d, e] via PE transposes.
        qT = p_qk.tile([128, ET * C], MF, tag="qT", name="qT")
        kTt = p_qk.tile([128, ET * C], MF, tag="kTt", name="kTt")
        for src, dst, nm in ((qw, qT, "q"), (kw, kTt, "k")):
            for ct in range(CT):
                for et in range(ET):
                    pt = p_tr.tile([128, 128], MF, tag="tr", name=f"tr{nm}{ct}_{et}")
                    nc.tensor.transpose(
                        pt[:], src[:, ct * C + et * 128: ct * C + (et + 1) * 128],
                        ident[:])
                    nc.any.tensor_copy(
                        dst[:, et * C + ct * 128: et * C + (ct + 1) * 128], pt[:])
        gmat = p_g.tile([128, CT * C], MF, tag="gmat", name="gmat")
        for ct in range(CT):
            ps = p_mm.tile([128, C], F32, tag="mm", name=f"psg{ct}")
            for et in range(ET):
                nc.tensor.matmul(
                    ps[:],
                    qT[:, et * C + ct * 128: et * C + (ct + 1) * 128],
                    kTt[:, et * C:(et + 1) * C],
                    start=(et == 0), stop=(et == ET - 1))
            nc.scalar.mul(gmat[:, ct * C:(ct + 1) * C], ps[:], ISQ)
        p_qk.release()
        keepwarm(xt[0][:], n=8)
        keepwarm(xt[1][:], n=8)

        # ---- scale/bias from global stats (wide [128, CT] ops) ---------
        epsb = p_small.tile([128, 1], F32, tag="epsb", name="epsb")
        nc.vector.memset(epsb[:], EPS)
        mean = p_small.tile([128, CT], F32, tag="mean", name="mean")
        nc.scalar.mul(mean[:], statsg[:, 0:CT], 1.0 / NB)
        ex2 = p_small.tile([128, CT], F32, tag="ex2", name="ex2")
        nc.scalar.mul(ex2[:], statsg[:, CT:2 * CT], 1.0 / NB)
        var = p_small.tile([128, CT], F32, tag="var", name="var")
        nc.vector.scalar_tensor_tensor(var[:], mean[:], 1.0, mean[:],
                                       OP.mult, OP.mult)
        nc.vector.tensor_tensor(var[:], ex2[:], var[:], OP.subtract)
        std = p_small.tile([128, CT], F32, tag="std", name="std")
        nc.scalar.activation(std[:], var[:], ACT.Sqrt, bias=epsb[:])
        scale = p_small.tile([128, CT], F32, tag="scale", name="scale")
        nc.vector.reciprocal(scale[:], std[:])
        nbias = p_small.tile([128, CT], F32, tag="nbias", name="nbias")
        nc.vector.scalar_tensor_tensor(nbias[:], mean[:], -1.0, scale[:],
                                       OP.mult, OP.mult)

        # ---- xn = x*scale + nbias (split scalar/vector) ----------------
        p_xn = tc.alloc_tile_pool(name="xn", bufs=1, side="right")
        xn = [p_xn.tile([128, CT * S], MF, tag=f"xn{b}", name=f"xn{b}")
              for b in range(b_loc)]
        for b in range(b_loc):
            for ct in range(CT):
                osl = xn[b][:, ct * S:(ct + 1) * S]
                isl = xt[b][:, ct * S:(ct + 1) * S]
                if (b * CT + ct) % 2 == 0:
                    nc.scalar.activation(osl, isl, ACT.Identity,
                                         bias=nbias[:, ct:ct + 1],
                                         scale=scale[:, ct:ct + 1])
                else:
                    nc.vector.tensor_scalar(osl, isl, scale[:, ct:ct + 1],
                                            nbias[:, ct:ct + 1],
                                            OP.mult, OP.add)
        keepwarm(xn[0][:], n=2)
        p_x.release()

        # ---- yT[d, s] = sum_c G[c, d] xn[c, s] -------------------------
        p_yt = tc.alloc_tile_pool(name="yt", bufs=1, side="right")
        yT = [p_yt.tile([128, DT * S], MF, tag=f"yT{b}", name=f"yT{b}")
              for b in range(b_loc)]
        # sh (q-range) outermost: all dt tiles of chunk-0's columns finish
        # first so the scores0 matmuls can start while chunk-1's yT runs
        for sh in range(NSH):
            pss = [[p_mm.tile([128, WS], F32, tag="mm", name=f"psy{b}_{dt}_{sh}")
                    for dt in range(DT)] for b in range(b_loc)]
            for dt in range(DT):
                for ct in range(CT):
                    for b in range(b_loc):
                        nc.tensor.matmul(
                            pss[b][dt][:],
                            gmat[:, ct * C + dt * 128: ct * C + (dt + 1) * 128],
                            xn[b][:, ct * S + sh * WS: ct * S + (sh + 1) * WS],
                            start=(ct == 0), stop=(ct == CT - 1))
                for b in range(b_loc):
                    nc.any.tensor_copy(
                        yT[b][:, dt * S + sh * WS: dt * S + (sh + 1) * WS],
                        pss[b][dt][:])

        # ---- scores + softmax pipeline ---------------------------------
        p_sc = tc.alloc_tile_pool(name="scores", bufs=1)
        p_soft = tc.alloc_tile_pool(name="soft", bufs=1)
        scT = [[p_sc.tile([128, ST * QC], F32, tag=f"sc{g}_{b}", name=f"sc{g}_{b}")
                for b in range(b_loc)] for g in range(G)]
        mq = [p_soft.tile([128, ST * QC], MAXDT, tag="mqbuf", name=f"mq{g}",
                          bufs=1) for g in range(G)]
        mgs = []

        def scores_chunk(g):
            for kt in range(ST):
                for b in range(b_loc):
                    for qh in range(QC // WQ):
                        ps = p_mm.tile([128, WQ], F32, tag="mm",
                                       name=f"pss{g}_{kt}_{b}_{qh}")
                        for dt in range(DT):
                            nc.tensor.matmul(
                                ps[:],
                                xn[b][:, dt * S + kt * 128: dt * S + (kt + 1) * 128],
                                yT[b][:, dt * S + g * QC + qh * WQ: dt * S + g * QC + (qh + 1) * WQ],
                                start=(dt == 0), stop=(dt == DT - 1))
                        nc.scalar.copy(
                            scT[g][b][:, kt * QC + qh * WQ: kt * QC + (qh + 1) * WQ],
                            ps[:])
                ksl = slice(kt * QC, (kt + 1) * QC)
                mscr = p_soft.tile([128, QC], F32, tag="mscr",
                                   name=f"mscr{g}_{kt}", bufs=2)
                nc.vector.tensor_tensor(mscr[:], scT[g][0][:, ksl],
                                        scT[g][1][:, ksl], OP.max)
                if use_int8_max:
                    nc.vector.tensor_scalar_mul(mq[g][:, ksl], mscr[:],
                                                1.0 / MQ_SCALE)
                else:
                    nc.vector.tensor_copy(mq[g][:, ksl], mscr[:])
            nc.sync.dma_start(out=max_in[g][:, :], in_=mq[g][:])
            keepwarm(yT[0][:], n=2)
            nc.gpsimd.collective_compute(
                "AllReduce", OP.max, replica_groups=rg,
                ins=[max_in[g].ap().opt()], outs=[max_out[g].ap().opt()])

        scores_chunk(0)
        scores_chunk(1)
        p_yt.release()

        zgs = []

        def softmax_chunk(g):
            # load the max-AR result on the vector queue: the first consumer
            # (the subtract) is a vector op anyway, so no queue is blocked
            mg = p_soft.tile([128, ST * QC], MAXDT, tag="mgbuf", name=f"mg{g}",
                             bufs=1)
            nc.gpsimd.dma_start(out=mg[:], in_=max_out[g][:, :])
            keepwarm(scT[g][0][:], n=2)
            zloc = p_soft.tile([128, ST * QC], BF16, tag="zloc", name=f"zloc{g}",
                               bufs=1)
            for kt2 in range(ST // 2):
                ksl = slice(kt2 * 2 * QC, (kt2 + 1) * 2 * QC)
                for b in range(b_loc):
                    if use_int8_max:
                        nc.vector.scalar_tensor_tensor(
                            scT[g][b][:, ksl], scT[g][b][:, ksl], 1.0 / MQ_SCALE,
                            mg[:, ksl], OP.mult, OP.subtract)
                        nc.scalar.activation(scT[g][b][:, ksl], scT[g][b][:, ksl],
                                             ACT.Exp, scale=MQ_SCALE)
                    else:
                        nc.vector.scalar_tensor_tensor(
                            scT[g][b][:, ksl], scT[g][b][:, ksl], 1.0,
                            mg[:, ksl], OP.mult, OP.subtract)
                        nc.scalar.activation(scT[g][b][:, ksl], scT[g][b][:, ksl],
                                             ACT.Exp)
                nc.vector.tensor_tensor(zloc[:, ksl], scT[g][0][:, ksl],
                                        scT[g][1][:, ksl], OP.add)
            nc.sync.dma_start(out=sum_in[g][:, :], in_=zloc[:])
            nc.gpsimd.collective_compute(
                "AllReduce", OP.add, replica_groups=rg,
                ins=[sum_in[g].ap().opt()], outs=[sum_out[g].ap().opt()])
            keepwarm(zloc[:], n=2)

        softmax_chunk(0)

        # ---- v = xn @ V (bf16), scheduled under the AR windows ---------
        p_v = tc.alloc_tile_pool(name="vv", bufs=1)
        vt = [p_v.tile([128, ST * C], BF16, tag=f"v{b}", name=f"v{b}")
              for b in range(b_loc)]
        for b in range(b_loc):
            for st in range(ST):
                ps = p_mm.tile([128, C], F32, tag="mm", name=f"psv{b}_{st}")
                for ct in range(CT):
                    nc.tensor.matmul(
                        ps[:],
                        xn[b][:, ct * S + st * 128: ct * S + (st + 1) * 128],
                        vw[:, ct * C: (ct + 1) * C],
                        start=(ct == 0), stop=(ct == CT - 1))
                nc.scalar.copy(vt[b][:, st * C:(st + 1) * C], ps[:])

        softmax_chunk(1)
        keepwarm(scT[1][0][:], n=6)
        # ---- xnT (bf16) via PE transposes (no DMA: slow strided DMA
        # transposes poison the shared DMA-completion semaphores that the
        # collective triggers wait on) ---------------------------------
        xnT = [p_v.tile([128, ST * C], BF16, tag=f"xnT{b}", name=f"xnT{b}")
               for b in range(b_loc)]
        for b in range(b_loc):
            for ct in range(CT):
                for st in range(ST):
                    pt = p_tr.tile([128, 128], MF, tag="tr",
                                   name=f"trx{b}_{ct}_{st}")
                    nc.tensor.transpose(
                        pt[:],
                        xn[b][:, ct * S + st * 128: ct * S + (st + 1) * 128],
                        ident[:])
                    dst = xnT[b][:, st * C + ct * 128: st * C + (ct + 1) * 128]
                    if (ct + st) % 2 == 0:
                        nc.vector.tensor_copy(dst, pt[:])
                    else:
                        nc.scalar.copy(dst, pt[:])

        p_xn.release()
        p_g.release()
        p_vw.release()

        # ---- normalize, attention, residual, store ---------------------
        p_att = tc.alloc_tile_pool(name="psum_attn", bufs=2, space="PSUM")
        p_rz = tc.alloc_tile_pool(name="rz", bufs=1, side="right")
        p_out = tc.alloc_tile_pool(name="outs", bufs=2, side="right")

        def attn_chunk(g):
            zg = p_soft.tile([128, ST * QC], BF16, tag="zgbuf", name=f"zg{g}",
                             bufs=1)
            half = ST * QC // 2
            nc.gpsimd.dma_start(out=zg[:, :half], in_=sum_out[g][:, :half])
            nc.gpsimd.dma_start(out=zg[:, half:], in_=sum_out[g][:, half:])
            zgs.append(zg)
            keepwarm(zgs[g][:], n=2)
            rz = p_rz.tile([128, ST * QC], F32, tag="rz", name=f"rz{g}")
            wb = [p_soft.tile([128, ST * QC], BF16, tag="wbbuf",
                              name=f"wb{g}_{b}", bufs=2) for b in range(b_loc)]
            # sliced so the first attn matmuls can start as soon as the first
            # kt-group of weights is normalized
            for kth in range(ST // 2):
                ksl = slice(kth * 2 * QC, (kth + 1) * 2 * QC)
                zgf = p_rz.tile([128, 2 * QC], F32, tag="zgf",
                                name=f"zgf{g}_{kth}", bufs=2)
                nc.scalar.copy(zgf[:], zg[:, ksl])
                nc.vector.reciprocal_approx_fast(out=rz[:, ksl], in_=zgf[:])
                for b in range(b_loc):
                    nc.vector.tensor_tensor(wb[b][:, ksl], scT[g][b][:, ksl],
                                            rz[:, ksl], OP.mult)
            for b in range(b_loc):
                ot = p_out.tile([128, QT * C], F32, tag="ot", name=f"ot{g}_{b}")
                for qt in range(QT):
                    st_glob = g * QT + qt
                    pa = p_att.tile([128, C], F32, tag="attn", name=f"pa{g}_{b}_{qt}")
                    for kt in range(ST):
                        wsl = wb[b][:, kt * QC + qt * 128: kt * QC + (qt + 1) * 128]
                        nc.tensor.matmul(
                            pa[:], wsl, vt[b][:, kt * C:(kt + 1) * C],
                            start=(kt == 0), stop=(kt == ST - 1))
                    nc.vector.scalar_tensor_tensor(
                        ot[:, qt * C:(qt + 1) * C], pa[:], 1.0,
                        xnT[b][:, st_glob * C:(st_glob + 1) * C],
                        OP.mult, OP.add)
                    nc.sync.dma_start(
                        out=out_ext[b, g * QC + qt * 128: g * QC + (qt + 1) * 128, :],
                        in_=ot[:, qt * C:(qt + 1) * C])

        attn_chunk(0)
        attn_chunk(1)

        for p in (p_out, p_rz, p_att, p_v, p_soft, p_sc, p_small, p_tr, p_mm):
            p.release()

    nc.compile()
    return nc


def _round_f32r(a):
    """Round-to-nearest-even to the FP32R encoding (11-bit mantissa)."""
    bits = np.ascontiguousarray(a, dtype=np.float32).view(np.uint32).copy()
    low = bits & np.uint32(0xFFF)
    bit12 = (bits >> np.uint32(12)) & np.uint32(1)
    up = (low > 0x800) | ((low == 0x800) & (bit12 == 1))
    bits = (bits & ~np.uint32(0xFFF)) + (up.astype(np.uint32) << np.uint32(12))
    return bits.view(np.float32)


def make_runner(nc, n_cores):
    install_neuronx_cc_hook()
    partition_name = nc.partition_id_tensor.name if nc.partition_id_tensor else None
    in_names, out_names, out_avals, zero_shapes = [], [], [], []
    for alloc in nc.m.functions[0].allocations:
        if not isinstance(alloc, mybir.MemoryLocationSet):
            continue
        name = alloc.memorylocations[0].name
        if alloc.kind == "ExternalInput":
            if name != partition_name:
                in_names.append(name)
        elif alloc.kind == "ExternalOutput":
            out_names.append(name)
            shape = tuple(alloc.tensor_shape)
            dtype = mybir.dt.np(alloc.dtype)
            out_avals.append(jax.core.ShapedArray(shape, dtype))
            zero_shapes.append((shape, dtype))
    n_params = len(in_names)
    n_outs = len(out_avals)
    all_in_names = list(in_names) + list(out_names)
    if partition_name is not None:
        all_in_names.append(partition_name)

    def _body(*args):
        operands = list(args)
        if partition_name is not None:
            operands.append(bass2jax.partition_id_tensor())
        outs = _bass_exec_p.bind(
            *operands,
            out_avals=tuple(out_avals),
            in_names=tuple(all_in_names),
            out_names=tuple(out_names),
            lowering_input_output_aliases=(),
            sim_require_finite=True,
            sim_require_nnan=True,
            nc=nc,
        )
        return tuple(outs)

    devices = jax.devices()[:n_cores]
    mesh = Mesh(np.asarray(devices), ("core",))
    in_specs = (PartitionSpec("core"),) * (n_params + n_outs)
    out_specs = (PartitionSpec("core"),) * n_outs
    sharded = jax.jit(
        shard_map(_body, mesh=mesh, in_specs=in_specs, out_specs=out_specs,
                  check_rep=False),
        donate_argnums=tuple(range(n_params, n_params + n_outs)),
        keep_unused=True,
    )
    shd = NamedSharding(mesh, PartitionSpec("core"))

    def run(in_maps):
        concat_in = [
            np.concatenate([np.asarray(in_maps[c][name]) for c in range(n_cores)],
                           axis=0)
            for name in in_names
        ]
        concat_zeros = [
            np.zeros((n_cores * s[0], *s[1:]), d) for (s, d) in zero_shapes
        ]
        # pre-place on devices, wait for transfers to complete
        placed = [jax.device_put(a, shd) for a in concat_in + concat_zeros]
        jax.block_until_ready(placed)
        out_arrs = sharded(*placed)
        jax.block_until_ready(out_arrs)
        return [
            {name: np.asarray(out_arrs[i]).reshape(n_cores, *out_avals[i].shape)[c]
             for i, name in enumerate(out_names)}
            for c in range(n_cores)
        ]

    return run


_CACHE = {}


def kernel(x, Q, K, V):
    B, C, H, W = x.shape
    S = H * W
    b_loc = B // N_CORES
    key = (B, C, S)
    if key not in _CACHE:
        nc = build_nc(C=C, S=S, b_loc=b_loc)
        _CACHE[key] = make_runner(nc, N_CORES)
    run = _CACHE[key]

    xr = np.ascontiguousarray(np.asarray(x, dtype=np.float32).reshape(B, C, S))
    Qr, Kr, Vr = _round_f32r(Q), _round_f32r(K), _round_f32r(V)
    in_maps = [{"x": xr[b_loc * i: b_loc * (i + 1)], "Q": Qr, "K": Kr, "V": Vr}
               for i in range(N_CORES)]
    results = run(in_maps)
    out = np.concatenate(
        [results[i]["out"].reshape(b_loc, S, C) for i in range(N_CORES)], axis=0)
    return out.astype(np.float32)


# revision 25
# speedup vs baseline: 1.0473x; 1.0236x over previous
"""Self-contained Trainium2 kernel for nn_AttentionEntryPoint (dense attention
with BatchNorm input norm and softmax over the batch axis), batch-sharded
across 8 NeuronCores.

Key structure (v2):
- scores = xn (Q K^T/sqrt(C)) xn^T: G = Q K^T is precomputed on the PE during
  the collective entry-barrier / stats-AllReduce stall (no stats dependency),
  so only xn -> yT = G-projection -> scores(chunk0) sit on the critical path
  before the first big AllReduce trigger. The separate q/k projections of the
  baseline are gone.
- The cross-batch softmax max is AllReduced as int8 (scores/32 rounded):
  any consistent shift within ~85 of the true max is mathematically exact for
  softmax and safe for exp range, and |scores| < 4064 fits int8*32. This
  halves the max-AR bytes. Dequant is folded into exp via activation scale.
- v, xnT and the second q-chunk work are scheduled under the AllReduce
  windows where the PE would otherwise idle.

kernel(**inputs) takes the full inputs (x [16,512,32,32], Q/K/V [512,512])
and returns the full output [16, 1024, 512] (attn + xn, as in the reference).
"""
import sys
import numpy as np

for _p in ("/opt/trn_rl_repo",):
    if _p not in sys.path:
        sys.path.append(_p)

import jax
from jax.sharding import Mesh, PartitionSpec, NamedSharding
from jax.experimental.shard_map import shard_map

import concourse.bass as bass
import concourse.bacc as bacc
import concourse.mybir as mybir
import concourse.tile as tile
from concourse import bass2jax
from concourse.bass2jax import _bass_exec_p, install_neuronx_cc_hook
from concourse.masks import make_identity

F32 = mybir.dt.float32
F32R = mybir.dt.float32r
BF16 = mybir.dt.bfloat16
I8 = mybir.dt.int8
AX = mybir.AxisListType
OP = mybir.AluOpType
ACT = mybir.ActivationFunctionType

N_CORES = 8
EPS = 1e-5
MQ_SCALE = 32.0  # int8 max quantization step; covers |scores| <= 4064


def build_nc(C=512, S=1024, b_loc=2, num_cores=N_CORES, warmup=16, g_chunks=2,
             use_int8_max=True):
    CT = C // 128          # channel tiles (contraction side of projections)
    DT = C // 128          # d tiles
    ST = S // 128          # s tiles
    ET = C // 128          # e tiles (contraction of G = Q K^T)
    G = g_chunks
    QC = S // G            # q chunk width
    QT = QC // 128         # q tiles per chunk
    NB = num_cores * b_loc * S   # batchnorm sample count (B*S)
    WS = min(512, S)             # matmul moving width for S-wide passes
    WQ = min(512, QC)            # matmul moving width within a q chunk
    NSH = S // WS
    ISQ = float(1.0 / np.sqrt(np.float32(C)))
    MF = F32R
    MAXDT = I8 if use_int8_max else BF16

    nc = bacc.Bacc("TRN2", target_bir_lowering=False, num_devices=num_cores)
    rg = [list(range(num_cores))]

    x_ext = nc.dram_tensor("x", [b_loc, C, S], F32, kind="ExternalInput").ap()
    q_ext = nc.dram_tensor("Q", [C, C], MF, kind="ExternalInput").ap()
    k_ext = nc.dram_tensor("K", [C, C], MF, kind="ExternalInput").ap()
    v_ext = nc.dram_tensor("V", [C, C], MF, kind="ExternalInput").ap()
    out_ext = nc.dram_tensor("out", [b_loc, S, C], F32, kind="ExternalOutput").ap()

    stats_in = nc.dram_tensor("stats_in", [128, 2 * CT], F32)
    stats_out = nc.dram_tensor("stats_out", [128, 2 * CT], F32, addr_space="Shared")
    max_in = [nc.dram_tensor(f"max_in{g}", [128, ST * QC], MAXDT) for g in range(G)]
    max_out = [nc.dram_tensor(f"max_out{g}", [128, ST * QC], MAXDT,
                              addr_space="Shared") for g in range(G)]
    sum_in = [nc.dram_tensor(f"sum_in{g}", [128, ST * QC], BF16) for g in range(G)]
    sum_out = [nc.dram_tensor(f"sum_out{g}", [128, ST * QC], BF16,
                              addr_space="Shared") for g in range(G)]

    with tile.TileContext(nc) as tc:
        p_small = tc.alloc_tile_pool(name="small", bufs=1)
        p_x = tc.alloc_tile_pool(name="xdata", bufs=1)
        p_vw = tc.alloc_tile_pool(name="vw", bufs=1, side="right")
        p_g = tc.alloc_tile_pool(name="gmat", bufs=1, side="right")
        p_qk = tc.alloc_tile_pool(name="qkw", bufs=1, side="right")
        p_mm = tc.alloc_tile_pool(name="psum_mm", bufs=4, space="PSUM")
        p_tr = tc.alloc_tile_pool(name="psum_tr", bufs=2, space="PSUM")

        kwseq = [0]

        def keepwarm(dep, n=2):
            # dummy matmuls reading `dep` to keep the PE HAM clock warm
            w = min(512, dep.shape[1])
            for i in range(n):
                kwseq[0] += 1
                pw = p_mm.tile([128, w], F32, tag="mm", name=f"kw{kwseq[0]}")
                nc.tensor.matmul(pw[:], dep[:, :128], dep[:, :w],
                                 start=True, stop=True)

        # ---- constants / warmup ----------------------------------------
        identf = p_small.tile([128, 128], F32, tag="identf", name="identf")
        make_identity(nc, identf[:])
        ident = p_small.tile([128, 128], MF, tag="ident", name="ident")
        nc.vector.tensor_copy(ident[:], identf[:])
        for i in range(warmup):
            pw = p_mm.tile([128, 128], F32, tag="mm", name=f"wup{i}")
            nc.tensor.matmul(pw[:], (ident[:]), (ident[:]),
                             start=True, stop=True)

        # ---- loads (x first: stats path is latency-critical) -----------
        xt = [p_x.tile([128, CT * S], F32, tag=f"x{b}", name=f"x{b}")
              for b in range(b_loc)]
        for b in range(b_loc):
            for ct in range(CT):
                nc.sync.dma_start(
                    out=xt[b][:, ct * S:(ct + 1) * S],
                    in_=x_ext[b, ct * 128:(ct + 1) * 128, :])
        qw = p_qk.tile([128, CT * C], MF, tag="qw", name="qw")
        kw = p_qk.tile([128, CT * C], MF, tag="kw", name="kw")
        vw = p_vw.tile([128, CT * C], MF, tag="vw", name="vw")
        for wt, ext in ((qw, q_ext), (kw, k_ext), (vw, v_ext)):
            nc.sync.dma_start(
                out=wt[:].rearrange("p (ct d) -> p ct d", ct=CT),
                in_=ext.rearrange("(ct p) d -> p ct d", p=128))

        # ---- batchnorm stats + AllReduce -------------------------------
        stats = p_small.tile([128, 2 * CT], F32, tag="stats", name="stats")
        xsum = [[p_small.tile([128, 1], F32, tag=f"xs{b}_{ct}", name=f"xs{b}_{ct}")
                 for ct in range(CT)] for b in range(b_loc)]
        xsq = [[p_small.tile([128, 1], F32, tag=f"xq{b}_{ct}", name=f"xq{b}_{ct}")
                for ct in range(CT)] for b in range(b_loc)]
        p_scr = tc.alloc_tile_pool(name="sqscr", bufs=2)
        for b in range(b_loc):
            for ct in range(CT):
                xs = xt[b][:, ct * S:(ct + 1) * S]
                nc.vector.tensor_reduce(xsum[b][ct][:], xs, AX.X, OP.add)
                sq = p_scr.tile([128, S], F32, tag="sqscr", name=f"sq{b}_{ct}")
                nc.scalar.activation(sq[:], xs, ACT.Square, accum_out=xsq[b][ct][:])
        p_scr.release()
        for ct in range(CT):
            nc.vector.tensor_tensor(stats[:, ct:ct + 1],
                                    xsum[0][ct][:], xsum[1][ct][:], OP.add)
            nc.vector.tensor_tensor(stats[:, CT + ct:CT + ct + 1],
                                    xsq[0][ct][:], xsq[1][ct][:], OP.add)
        nc.sync.dma_start(out=stats_in[:, :], in_=stats[:])
        nc.gpsimd.collective_compute(
            "AllReduce", OP.add, replica_groups=rg,
            ins=[stats_in.ap().opt()], outs=[stats_out.ap().opt()])
        statsg = p_small.tile([128, 2 * CT], F32, tag="statsg", name="statsg")
        nc.sync.dma_start(out=statsg[:], in_=stats_out[:, :])

        # ---- G = (Q K^T)/sqrt(C) during the stats stall ----------------
        # QT[e, c] = Q[c, e], KT[e, d] = K[d, e] via PE transposes.
        qT = p_qk.tile([128, ET * C], MF, tag="qT", name="qT")
        kTt = p_qk.tile([128, ET * C], MF, tag="kTt", name="kTt")
        for src, dst, nm in ((qw, qT, "q"), (kw, kTt, "k")):
            for ct in range(CT):
                for et in range(ET):
                    pt = p_tr.tile([128, 128], MF, tag="tr", name=f"tr{nm}{ct}_{et}")
                    nc.tensor.transpose(
                        pt[:], src[:, ct * C + et * 128: ct * C + (et + 1) * 128],
                        ident[:])
                    nc.any.tensor_copy(
                        dst[:, et * C + ct * 128: et * C + (ct + 1) * 128], pt[:])
        gmat = p_g.tile([128, CT * C], MF, tag="gmat", name="gmat")
        for ct in range(CT):
            ps = p_mm.tile([128, C], F32, tag="mm", name=f"psg{ct}")
            for et in range(ET):
                nc.tensor.matmul(
                    ps[:],
                    qT[:, et * C + ct * 128: et * C + (ct + 1) * 128],
                    kTt[:, et * C:(et + 1) * C],
                    start=(et == 0), stop=(et == ET - 1))
            nc.scalar.mul(gmat[:, ct * C:(ct + 1) * C], ps[:], ISQ)
        p_qk.release()
        keepwarm(xt[0][:], n=8)
        keepwarm(xt[1][:], n=8)

        # ---- scale/bias from global stats (wide [128, CT] ops) ---------
        epsb = p_small.tile([128, 1], F32, tag="epsb", name="epsb")
        nc.vector.memset(epsb[:], EPS)
        mean = p_small.tile([128, CT], F32, tag="mean", name="mean")
        nc.scalar.mul(mean[:], statsg[:, 0:CT], 1.0 / NB)
        ex2 = p_small.tile([128, CT], F32, tag="ex2", name="ex2")
        nc.scalar.mul(ex2[:], statsg[:, CT:2 * CT], 1.0 / NB)
        var = p_small.tile([128, CT], F32, tag="var", name="var")
        nc.vector.scalar_tensor_tensor(var[:], mean[:], 1.0, mean[:],
                                       OP.mult, OP.mult)
        nc.vector.tensor_tensor(var[:], ex2[:], var[:], OP.subtract)
        std = p_small.tile([128, CT], F32, tag="std", name="std")
        nc.scalar.activation(std[:], var[:], ACT.Sqrt, bias=epsb[:])
        scale = p_small.tile([128, CT], F32, tag="scale", name="scale")
        nc.vector.reciprocal(scale[:], std[:])
        nbias = p_small.tile([128, CT], F32, tag="nbias", name="nbias")
        nc.vector.scalar_tensor_tensor(nbias[:], mean[:], -1.0, scale[:],
                                       OP.mult, OP.mult)

        # ---- xn = x*scale + nbias (split scalar/vector) ----------------
        p_xn = tc.alloc_tile_pool(name="xn", bufs=1, side="right")
        xn = [p_xn.tile([128, CT * S], MF, tag=f"xn{b}", name=f"xn{b}")
              for b in range(b_loc)]
        for b in range(b_loc):
            for ct in range(CT):
                osl = xn[b][:, ct * S:(ct + 1) * S]
                isl = xt[b][:, ct * S:(ct + 1) * S]
                if (b * CT + ct) % 2 == 0:
                    nc.scalar.activation(osl, isl, ACT.Identity,
                                         bias=nbias[:, ct:ct + 1],
                                         scale=scale[:, ct:ct + 1])
                else:
                    nc.vector.tensor_scalar(osl, isl, scale[:, ct:ct + 1],
                                            nbias[:, ct:ct + 1],
                                            OP.mult, OP.add)
        keepwarm(xn[0][:], n=2)
        p_x.release()

        # ---- yT[d, s] = sum_c G[c, d] xn[c, s] -------------------------
        p_yt = tc.alloc_tile_pool(name="yt", bufs=1, side="right")
        yT = [p_yt.tile([128, DT * S], MF, tag=f"yT{b}", name=f"yT{b}")
              for b in range(b_loc)]
        # sh (q-range) outermost: all dt tiles of chunk-0's columns finish
        # first so the scores0 matmuls can start while chunk-1's yT runs
        for sh in range(NSH):
            pss = [[p_mm.tile([128, WS], F32, tag="mm", name=f"psy{b}_{dt}_{sh}")
                    for dt in range(DT)] for b in range(b_loc)]
            for dt in range(DT):
                for ct in range(CT):
                    for b in range(b_loc):
                        nc.tensor.matmul(
                            pss[b][dt][:],
                            gmat[:, ct * C + dt * 128: ct * C + (dt + 1) * 128],
                            xn[b][:, ct * S + sh * WS: ct * S + (sh + 1) * WS],
                            start=(ct == 0), stop=(ct == CT - 1))
                for b in range(b_loc):
                    nc.any.tensor_copy(
                        yT[b][:, dt * S + sh * WS: dt * S + (sh + 1) * WS],
                        pss[b][dt][:])

        # ---- scores + softmax pipeline ---------------------------------
        p_sc = tc.alloc_tile_pool(name="scores", bufs=1)
        p_soft = tc.alloc_tile_pool(name="soft", bufs=1)
        scT = [[p_sc.tile([128, ST * QC], F32, tag=f"sc{g}_{b}", name=f"sc{g}_{b}")
                for b in range(b_loc)] for g in range(G)]
        mq = [p_soft.tile([128, ST * QC], MAXDT, tag="mqbuf", name=f"mq{g}",
                          bufs=1) for g in range(G)]
        mgs = []

        def scores_chunk(g):
            for kt in range(ST):
                for b in range(b_loc):
                    for qh in range(QC // WQ):
                        ps = p_mm.tile([128, WQ], F32, tag="mm",
                                       name=f"pss{g}_{kt}_{b}_{qh}")
                        for dt in range(DT):
                            nc.tensor.matmul(
                                ps[:],
                                xn[b][:, dt * S + kt * 128: dt * S + (kt + 1) * 128],
                                yT[b][:, dt * S + g * QC + qh * WQ: dt * S + g * QC + (qh + 1) * WQ],
                                start=(dt == 0), stop=(dt == DT - 1))
                        nc.scalar.copy(
                            scT[g][b][:, kt * QC + qh * WQ: kt * QC + (qh + 1) * WQ],
                            ps[:])
                ksl = slice(kt * QC, (kt + 1) * QC)
                mscr = p_soft.tile([128, QC], F32, tag="mscr",
                                   name=f"mscr{g}_{kt}", bufs=2)
                nc.vector.tensor_tensor(mscr[:], scT[g][0][:, ksl],
                                        scT[g][1][:, ksl], OP.max)
                if use_int8_max:
                    nc.vector.tensor_scalar_mul(mq[g][:, ksl], mscr[:],
                                                1.0 / MQ_SCALE)
                else:
                    nc.vector.tensor_copy(mq[g][:, ksl], mscr[:])
            nc.sync.dma_start(out=max_in[g][:, :], in_=mq[g][:])
            keepwarm(yT[0][:], n=2)
            nc.gpsimd.collective_compute(
                "AllReduce", OP.max, replica_groups=rg,
                ins=[max_in[g].ap().opt()], outs=[max_out[g].ap().opt()])

        scores_chunk(0)
        scores_chunk(1)
        p_yt.release()

        zgs = []

        def softmax_chunk(g):
            # load the max-AR result on the vector queue: the first consumer
            # (the subtract) is a vector op anyway, so no queue is blocked
            mg = p_soft.tile([128, ST * QC], MAXDT, tag="mgbuf", name=f"mg{g}",
                             bufs=1)
            nc.gpsimd.dma_start(out=mg[:], in_=max_out[g][:, :])
            keepwarm(scT[g][0][:], n=2)
            zloc = p_soft.tile([128, ST * QC], BF16, tag="zloc", name=f"zloc{g}",
                               bufs=1)
            for kt2 in range(ST // 2):
                ksl = slice(kt2 * 2 * QC, (kt2 + 1) * 2 * QC)
                for b in range(b_loc):
                    if use_int8_max:
                        nc.vector.scalar_tensor_tensor(
                            scT[g][b][:, ksl], scT[g][b][:, ksl], 1.0 / MQ_SCALE,
                            mg[:, ksl], OP.mult, OP.subtract)
                        nc.scalar.activation(scT[g][b][:, ksl], scT[g][b][:, ksl],
                                             ACT.Exp, scale=MQ_SCALE)
                    else:
                        nc.vector.scalar_tensor_tensor(
                            scT[g][b][:, ksl], scT[g][b][:, ksl], 1.0,
                            mg[:, ksl], OP.mult, OP.subtract)
                        nc.scalar.activation(scT[g][b][:, ksl], scT[g][b][:, ksl],
                                             ACT.Exp)
                nc.vector.tensor_tensor(zloc[:, ksl], scT[g][0][:, ksl],
                                        scT[g][1][:, ksl], OP.add)
            nc.sync.dma_start(out=sum_in[g][:, :], in_=zloc[:])
            nc.gpsimd.collective_compute(
                "AllReduce", OP.add, replica_groups=rg,
                ins=[sum_in[g].ap().opt()], outs=[sum_out[g].ap().opt()])
            keepwarm(zloc[:], n=2)

        softmax_chunk(0)

        softmax_chunk(1)
        keepwarm(scT[1][0][:], n=6)
        # ---- v = xn @ V (bf16), scheduled under the AR windows ---------
        p_v = tc.alloc_tile_pool(name="vv", bufs=1)
        vt = [p_v.tile([128, ST * C], BF16, tag=f"v{b}", name=f"v{b}")
              for b in range(b_loc)]
        for b in range(b_loc):
            for st in range(ST):
                ps = p_mm.tile([128, C], F32, tag="mm", name=f"psv{b}_{st}")
                for ct in range(CT):
                    nc.tensor.matmul(
                        ps[:],
                        xn[b][:, ct * S + st * 128: ct * S + (st + 1) * 128],
                        vw[:, ct * C: (ct + 1) * C],
                        start=(ct == 0), stop=(ct == CT - 1))
                if (b + st) % 2 == 0:
                    nc.scalar.copy(vt[b][:, st * C:(st + 1) * C], ps[:])
                else:
                    nc.vector.tensor_copy(vt[b][:, st * C:(st + 1) * C], ps[:])

        # ---- xnT (bf16) via PE transposes (no DMA: slow strided DMA
        # transposes poison the shared DMA-completion semaphores that the
        # collective triggers wait on) ---------------------------------
        xnT = [p_v.tile([128, ST * C], BF16, tag=f"xnT{b}", name=f"xnT{b}")
               for b in range(b_loc)]
        for b in range(b_loc):
            for ct in range(CT):
                for st in range(ST):
                    pt = p_tr.tile([128, 128], MF, tag="tr",
                                   name=f"trx{b}_{ct}_{st}")
                    nc.tensor.transpose(
                        pt[:],
                        xn[b][:, ct * S + st * 128: ct * S + (st + 1) * 128],
                        ident[:])
                    dst = xnT[b][:, st * C + ct * 128: st * C + (ct + 1) * 128]
                    if (ct + st) % 2 == 0:
                        nc.vector.tensor_copy(dst, pt[:])
                    else:
                        nc.scalar.copy(dst, pt[:])

        p_xn.release()
        p_g.release()
        p_vw.release()

        # ---- normalize, attention, residual, store ---------------------
        p_att = tc.alloc_tile_pool(name="psum_attn", bufs=2, space="PSUM")
        p_rz = tc.alloc_tile_pool(name="rz", bufs=1, side="right")
        p_out = tc.alloc_tile_pool(name="outs", bufs=2, side="right")

        def attn_chunk(g):
            zg = p_soft.tile([128, ST * QC], BF16, tag="zgbuf", name=f"zg{g}",
                             bufs=1)
            half = ST * QC // 2
            nc.gpsimd.dma_start(out=zg[:, :half], in_=sum_out[g][:, :half])
            nc.gpsimd.dma_start(out=zg[:, half:], in_=sum_out[g][:, half:])
            zgs.append(zg)
            keepwarm(zgs[g][:], n=2)
            rz = p_rz.tile([128, ST * QC], F32, tag="rz", name=f"rz{g}")
            wb = [p_soft.tile([128, ST * QC], BF16, tag="wbbuf",
                              name=f"wb{g}_{b}", bufs=2) for b in range(b_loc)]
            # sliced so the first attn matmuls can start as soon as the first
            # kt-group of weights is normalized
            for kth in range(ST // 2):
                ksl = slice(kth * 2 * QC, (kth + 1) * 2 * QC)
                zgf = p_rz.tile([128, 2 * QC], F32, tag="zgf",
                                name=f"zgf{g}_{kth}", bufs=2)
                nc.scalar.copy(zgf[:], zg[:, ksl])
                nc.vector.reciprocal_approx_fast(out=rz[:, ksl], in_=zgf[:])
                for b in range(b_loc):
                    nc.vector.tensor_tensor(wb[b][:, ksl], scT[g][b][:, ksl],
                                            rz[:, ksl], OP.mult)
            for b in range(b_loc):
                ot = p_out.tile([128, QT * C], F32, tag="ot", name=f"ot{g}_{b}")
                for qt in range(QT):
                    st_glob = g * QT + qt
                    pa = p_att.tile([128, C], F32, tag="attn", name=f"pa{g}_{b}_{qt}")
                    for kt in range(ST):
                        wsl = wb[b][:, kt * QC + qt * 128: kt * QC + (qt + 1) * 128]
                        nc.tensor.matmul(
                            pa[:], wsl, vt[b][:, kt * C:(kt + 1) * C],
                            start=(kt == 0), stop=(kt == ST - 1))
                    nc.vector.scalar_tensor_tensor(
                        ot[:, qt * C:(qt + 1) * C], pa[:], 1.0,
                        xnT[b][:, st_glob * C:(st_glob + 1) * C],
                        OP.mult, OP.add)
                    nc.sync.dma_start(
                        out=out_ext[b, g * QC + qt * 128: g * QC + (qt + 1) * 128, :],
                        in_=ot[:, qt * C:(qt + 1) * C])

        attn_chunk(0)
        # fill the PE gap while ARsum1 drains so attn1 starts at full clock
        keepwarm(vt[0][:], n=14)
        attn_chunk(1)

        for p in (p_out, p_rz, p_att, p_v, p_soft, p_sc, p_small, p_tr, p_mm):
            p.release()

    nc.compile()
    return nc


def _round_f32r(a):
    """Round-to-nearest-even to the FP32R encoding (11-bit mantissa)."""
    bits = np.ascontiguousarray(a, dtype=np.float32).view(np.uint32).copy()
    low = bits & np.uint32(0xFFF)
    bit12 = (bits >> np.uint32(12)) & np.uint32(1)
    up = (low > 0x800) | ((low == 0x800) & (bit12 == 1))
    bits = (bits & ~np.uint32(0xFFF)) + (up.astype(np.uint32) << np.uint32(12))
    return bits.view(np.float32)


def make_runner(nc, n_cores):
    install_neuronx_cc_hook()
    partition_name = nc.partition_id_tensor.name if nc.partition_id_tensor else None
    in_names, out_names, out_avals, zero_shapes = [], [], [], []
    for alloc in nc.m.functions[0].allocations:
        if not isinstance(alloc, mybir.MemoryLocationSet):
            continue
        name = alloc.memorylocations[0].name
        if alloc.kind == "ExternalInput":
            if name != partition_name:
                in_names.append(name)
        elif alloc.kind == "ExternalOutput":
            out_names.append(name)
            shape = tuple(alloc.tensor_shape)
            dtype = mybir.dt.np(alloc.dtype)
            out_avals.append(jax.core.ShapedArray(shape, dtype))
            zero_shapes.append((shape, dtype))
    n_params = len(in_names)
    n_outs = len(out_avals)
    all_in_names = list(in_names) + list(out_names)
    if partition_name is not None:
        all_in_names.append(partition_name)

    def _body(*args):
        operands = list(args)
        if partition_name is not None:
            operands.append(bass2jax.partition_id_tensor())
        outs = _bass_exec_p.bind(
            *operands,
            out_avals=tuple(out_avals),
            in_names=tuple(all_in_names),
            out_names=tuple(out_names),
            lowering_input_output_aliases=(),
            sim_require_finite=True,
            sim_require_nnan=True,
            nc=nc,
        )
        return tuple(outs)

    devices = jax.devices()[:n_cores]
    mesh = Mesh(np.asarray(devices), ("core",))
    in_specs = (PartitionSpec("core"),) * (n_params + n_outs)
    out_specs = (PartitionSpec("core"),) * n_outs
    sharded = jax.jit(
        shard_map(_body, mesh=mesh, in_specs=in_specs, out_specs=out_specs,
                  check_rep=False),
        donate_argnums=tuple(range(n_params, n_params + n_outs)),
        keep_unused=True,
    )
    shd = NamedSharding(mesh, PartitionSpec("core"))

    def run(in_maps):
        concat_in = [
            np.concatenate([np.asarray(in_maps[c][name]) for c in range(n_cores)],
                           axis=0)
            for name in in_names
        ]
        concat_zeros = [
            np.zeros((n_cores * s[0], *s[1:]), d) for (s, d) in zero_shapes
        ]
        # pre-place on devices, wait for transfers to complete
        placed = [jax.device_put(a, shd) for a in concat_in + concat_zeros]
        jax.block_until_ready(placed)
        out_arrs = sharded(*placed)
        jax.block_until_ready(out_arrs)
        return [
            {name: np.asarray(out_arrs[i]).reshape(n_cores, *out_avals[i].shape)[c]
             for i, name in enumerate(out_names)}
            for c in range(n_cores)
        ]

    return run


_CACHE = {}


def kernel(x, Q, K, V):
    B, C, H, W = x.shape
    S = H * W
    b_loc = B // N_CORES
    key = (B, C, S)
    if key not in _CACHE:
        nc = build_nc(C=C, S=S, b_loc=b_loc)
        _CACHE[key] = make_runner(nc, N_CORES)
    run = _CACHE[key]

    xr = np.ascontiguousarray(np.asarray(x, dtype=np.float32).reshape(B, C, S))
    Qr, Kr, Vr = _round_f32r(Q), _round_f32r(K), _round_f32r(V)
    in_maps = [{"x": xr[b_loc * i: b_loc * (i + 1)], "Q": Qr, "K": Kr, "V": Vr}
               for i in range(N_CORES)]
    results = run(in_maps)
    out = np.concatenate(
        [results[i]["out"].reshape(b_loc, S, C) for i in range(N_CORES)], axis=0)
    return out.astype(np.float32)
